# revision 8
# baseline (speedup 1.0000x reference)
"""CIN (Compressed Interaction Network) kernel for Trainium2, 8-core data parallel.

Math (per batch row b, embedding dim d — R = B*D independent rows):
  layer k: cur_k[m, (b,d)] = sum_{f,g} W_k[f*G+g, m] * x0[f,(b,d)] * x_{k}[g,(b,d)]
  output  = concat_k( sum_d cur_k )    -> [B, 384]

Device strategy (per core, batch-sharded B/8 = 256 -> R = 4096 rows):
  * Everything lives feature-on-partitions: cur_k^T [128, R] etc.
  * z_k^T [(f,g), R] is built k-tile by k-tile on DVE tensor_tensor (bf16 2x):
      z-tile_f = cur_{k-1}^T * bcast(x0^T[f, :])
    The broadcast tiles come from one DRAM->SBUF DMA per chunk with a
    0-stride partition AP (DVE cannot partition-broadcast; DMA can).
  * The (f,g) contraction is standard PSUM-accumulated matmuls with weight
    k-tiles stationary, so the f-sum is free.
  * Layer 0 uses the x (x) x symmetry: W0 is host-symmetrized to the upper
    triangle (k: 1521 -> 780, padded 896) and the two z factors are
    host-gathered index patterns of x^T (pure indexing, no arithmetic).
  * All DRAM operands are laid out chunk-major on the host so every DMA has
    large contiguous per-partition runs (big packets -> full DMA bandwidth).
"""

import sys

sys.path.insert(0, "/opt/trn_rl_repo")

import numpy as np
import ml_dtypes

import concourse.bass as bass
import concourse.mybir as mybir
from concourse import bacc
from concourse.tile import TileContext
from concourse.bass_utils import run_bass_kernel_spmd

BF16 = ml_dtypes.bfloat16

B, F0, D = 2048, 39, 16
M = 128                      # layer width (all three layers)
NCORES = 8
BPC = B // NCORES            # batch per core = 256
R = BPC * D                  # rows per core = 4096
K0 = (F0 * (F0 + 1)) // 2    # 780 (triangular)
K0P = 896                    # padded to 7 k-tiles
NKT0 = K0P // 128            # 7
NKT = (F0 * M) // 128        # 39 k-tiles for layers 1/2

L = 512                      # bd-chunk (32 b x 16 d)
NCHUNK = R // L              # 8
BPCH = L // D                # 32 batches per chunk

DT = mybir.dt.bfloat16
DTF = mybir.dt.float32

_CACHE = {}


def _build_program():
    nc = bacc.Bacc("TRN2", target_bir_lowering=False, debug=False,
                   num_devices=NCORES)

    # chunk-major layouts so per-partition DMA runs are contiguous
    xT = nc.declare_dram_parameter("xT", [NCHUNK, F0 * L], DT, isOutput=False)
    zin0 = nc.declare_dram_parameter("zin0", [NCHUNK, 128, NKT0, L], DT,
                                     isOutput=False)
    zin1 = nc.declare_dram_parameter("zin1", [NCHUNK, 128, NKT0, L], DT,
                                     isOutput=False)
    w0 = nc.declare_dram_parameter("w0", [K0P, M], DT, isOutput=False)
    w1 = nc.declare_dram_parameter("w1", [F0 * M, M], DT, isOutput=False)
    w2 = nc.declare_dram_parameter("w2", [F0 * M, M], DT, isOutput=False)
    ident = nc.declare_dram_parameter("ident", [128, 128], DTF, isOutput=False)
    out = nc.declare_dram_parameter("out", [BPC, 3 * M], DTF, isOutput=True)

    with TileContext(nc) as tc:
        with (
            tc.tile_pool(name="wpool", bufs=1) as wpool,
            tc.tile_pool(name="bcast", bufs=2) as bcpool,
            tc.tile_pool(name="zin", bufs=2) as zinpool,
            tc.tile_pool(name="zt", bufs=3) as zpool,
            tc.tile_pool(name="cur", bufs=2) as curpool,
            tc.tile_pool(name="outp", bufs=1) as outpool,
            tc.tile_pool(name="psum", bufs=4, space="PSUM") as pspool,
            tc.tile_pool(name="pst", bufs=2, space="PSUM") as pstpool,
        ):
            def issue_chunk_dmas(c, pieces=1):
                """Prefetch one chunk's layer-0 factors + broadcast set.
                pieces>1 splits the transfers so the first compute group can
                start before the whole chunk arrives (startup only)."""
                zin0t = zinpool.tile([128, NKT0, L], DT, tag="zin0",
                                     name=f"zin0_{c}")
                zin1t = zinpool.tile([128, NKT0, L], DT, tag="zin1",
                                     name=f"zin1_{c}")
                if pieces == 1:
                    nc.scalar.dma_start(out=zin0t[:], in_=zin0[c])
                    nc.scalar.dma_start(out=zin1t[:], in_=zin1[c])
                else:
                    kt = 0
                    for gsz in G0:
                        nc.scalar.dma_start(
                            out=zin0t[:, kt : kt + gsz, :],
                            in_=zin0[c, :, kt : kt + gsz, :],
                        )
                        nc.scalar.dma_start(
                            out=zin1t[:, kt : kt + gsz, :],
                            in_=zin1[c, :, kt : kt + gsz, :],
                        )
                        kt += gsz
                bc = bcpool.tile([128, F0, L], DT, tag="bc", name=f"bc_{c}")
                xTv = xT[c : c + 1, :].to_broadcast((128, F0 * L)).rearrange(
                    "p (f l) -> p f l", f=F0
                )
                if pieces == 1:
                    nc.gpsimd.dma_start(out=bc[:], in_=xTv)
                else:
                    f = 0
                    for gsz in G12:
                        nc.gpsimd.dma_start(
                            out=bc[:, f : f + gsz, :], in_=xTv[:, f : f + gsz, :]
                        )
                        f += gsz
                return bc, zin0t, zin1t

            G0 = [4, 3]          # layer-0 k-tile TT groups (sum NKT0)
            G12 = [8, 8, 8, 8, 7]  # layer-1/2 f-groups (sum F0)

            # ---- small weights first (L0's matmuls need w0 immediately),
            # then the chunk-0 prefetch; w1/w2 follow on the sync queue.
            w0s = wpool.tile([128, NKT0, M], DT, tag="w0")
            nc.sync.dma_start(out=w0s[:], in_=w0.rearrange("(t p) m -> p t m", p=128))
            ids = wpool.tile([128, 128], DTF, tag="ident")
            nc.sync.dma_start(out=ids[:], in_=ident[:])
            pref = issue_chunk_dmas(0, pieces=2)
            w1s = wpool.tile([128, NKT, M], DT, tag="w1")
            nc.sync.dma_start(out=w1s[:], in_=w1.rearrange("(t p) m -> p t m", p=128))
            w2s = wpool.tile([128, NKT, M], DT, tag="w2")
            nc.sync.dma_start(out=w2s[:], in_=w2.rearrange("(t p) m -> p t m", p=128))

            # per-layer output accumulators [128 m, BPC] fp32
            outacc = [
                outpool.tile([128, BPC], DTF, tag=f"oacc{k}", name=f"oacc{k}")
                for k in range(3)
            ]

            for c in range(NCHUNK):
                bc, zin0t, zin1t = pref
                if c + 1 < NCHUNK:
                    pref = issue_chunk_dmas(c + 1)

                # ---------- layer 0 (triangular) ----------
                ps0 = pspool.tile([128, L], DTF, tag="ps")
                kt = 0
                for gsz in G0:
                    z0t = zpool.tile([128, gsz, L], DT, tag="z")
                    nc.vector.tensor_mul(
                        z0t[:],
                        zin0t[:, kt : kt + gsz, :],
                        zin1t[:, kt : kt + gsz, :],
                    )
                    for j in range(gsz):
                        nc.tensor.matmul(
                            ps0[:],
                            w0s[:, kt + j, :],
                            z0t[:, j, :],
                            start=(kt + j == 0),
                            stop=(kt + j == NKT0 - 1),
                        )
                    kt += gsz

                cur0 = curpool.tile([128, L], DT, tag="cur0")
                nc.scalar.copy(cur0[:], ps0[:])
                nc.vector.tensor_reduce(
                    outacc[0][:, c * BPCH : (c + 1) * BPCH],
                    ps0[:].rearrange("p (b d) -> p b d", d=D),
                    axis=mybir.AxisListType.X,
                    op=mybir.AluOpType.add,
                )

                # ---------- layers 1 and 2 ----------
                prev = cur0
                for lyr, ws in ((1, w1s), (2, w2s)):
                    ps = pspool.tile([128, L], DTF, tag="ps")
                    f = 0
                    for gsz in G12:
                        zt = zpool.tile([128, gsz, L], DT, tag="z")
                        nc.vector.tensor_mul(
                            zt[:],
                            prev[:].unsqueeze(1).to_broadcast((128, gsz, L)),
                            bc[:, f : f + gsz, :],
                        )
                        for j in range(gsz):
                            nc.tensor.matmul(
                                ps[:],
                                ws[:, f + j, :],
                                zt[:, j, :],
                                start=(f + j == 0),
                                stop=(f + j == F0 - 1),
                            )
                        f += gsz

                    nc.vector.tensor_reduce(
                        outacc[lyr][:, c * BPCH : (c + 1) * BPCH],
                        ps[:].rearrange("p (b d) -> p b d", d=D),
                        axis=mybir.AxisListType.X,
                        op=mybir.AluOpType.add,
                    )
                    if lyr == 1:
                        cur1 = curpool.tile([128, L], DT, tag="cur1")
                        nc.scalar.copy(cur1[:], ps[:])
                        prev = cur1

            # ---------- transpose [128 m, BPC b] -> [BPC, 128] and store ----------
            for k in range(3):
                for h in range(BPC // 128):
                    pst = pstpool.tile([128, 128], DTF, tag="pst")
                    nc.tensor.transpose(
                        pst[:], outacc[k][:, h * 128 : (h + 1) * 128], ids[:]
                    )
                    ot = curpool.tile([128, 128], DTF, tag="otile")
                    nc.scalar.copy(ot[:], pst[:])
                    nc.sync.dma_start(
                        out=out[h * 128 : (h + 1) * 128, k * M : (k + 1) * M],
                        in_=ot[:],
                    )

    nc.compile()
    return nc


def _host_prep(inputs, f0, f1, f2):
    """Per-core input maps. Pure layout/cast/index-gather, no FLOP offload
    (except the W0 symmetrization, which is weight preprocessing)."""
    x = np.asarray(inputs)

    # symmetrized triangular W0: rows (f, g) f<=g
    f0n = np.asarray(f0).reshape(F0, F0, M)
    fi, gi = np.triu_indices(F0)
    w0t = f0n[fi, gi] + np.where((fi != gi)[:, None], f0n[gi, fi], 0.0)
    w0 = np.zeros((K0P, M), dtype=BF16)
    w0[:K0] = w0t.astype(BF16)

    w1 = np.asarray(f1).astype(BF16)
    w2 = np.asarray(f2).astype(BF16)
    ident = np.eye(128, dtype=np.float32)

    # layer-0 z-factor gather indices (triangular, k-row = tile*128 + p)
    pidx = np.arange(K0P)
    fidx = np.zeros(K0P, np.int64)
    gidx = np.zeros(K0P, np.int64)
    fidx[:K0], gidx[:K0] = fi, gi
    valid = (pidx < K0).astype(BF16)[:, None]

    maps = []
    for c in range(NCORES):
        xs = x[c * BPC : (c + 1) * BPC]                    # [256, 39, 16]
        xTf = np.ascontiguousarray(
            xs.transpose(1, 0, 2).reshape(F0, R)
        ).astype(BF16)                                     # [39, R]
        # chunk-major broadcast source: [NCHUNK, F0*L]
        xTc = np.ascontiguousarray(
            xTf.reshape(F0, NCHUNK, L).transpose(1, 0, 2)
        ).reshape(NCHUNK, F0 * L)
        # layer-0 factors [K0P, R] -> chunk-major [NCHUNK, 128, NKT0, L]
        z0a = (xTf[gidx] * valid).reshape(NKT0, 128, NCHUNK, L)
        z0b = (xTf[fidx] * valid).reshape(NKT0, 128, NCHUNK, L)
        zin0c = np.ascontiguousarray(z0a.transpose(2, 1, 0, 3))
        zin1c = np.ascontiguousarray(z0b.transpose(2, 1, 0, 3))
        maps.append(
            dict(xT=xTc, zin0=zin0c, zin1=zin1c, w0=w0, w1=w1, w2=w2,
                 ident=ident)
        )
    return maps


def kernel(**inputs) -> np.ndarray:
    if "nc" not in _CACHE:
        _CACHE["nc"] = _build_program()
    nc = _CACHE["nc"]
    maps = _host_prep(inputs["inputs"], inputs["f0"], inputs["f1"], inputs["f2"])
    res = run_bass_kernel_spmd(nc, maps, list(range(NCORES)))
    return np.concatenate([res.results[c]["out"] for c in range(NCORES)], axis=0)


if __name__ == "__main__":
    rng = np.random.default_rng(0)
    ins = {
        "inputs": rng.standard_normal((B, F0, D), dtype=np.float32),
        "f0": (rng.standard_normal((F0 * F0, M)) * 0.05).astype(np.float32),
        "f1": (rng.standard_normal((F0 * M, M)) * 0.05).astype(np.float32),
        "f2": (rng.standard_normal((F0 * M, M)) * 0.05).astype(np.float32),
    }
    out = kernel(**ins)
    print("out", out.shape, out.dtype)


# revision 9
# speedup vs baseline: 1.1206x; 1.1206x over previous
"""CIN (Compressed Interaction Network) kernel for Trainium2, 8-core data parallel.

Math (per batch row b, embedding dim d — R = B*D independent rows):
  layer k: cur_k[m, (b,d)] = sum_{f,g} W_k[f*G+g, m] * x0[f,(b,d)] * x_{k}[g,(b,d)]
  output  = concat_k( sum_d cur_k )    -> [B, 384]

Device strategy (per core, batch-sharded B/8 = 256 -> R = 4096 rows):
  * Everything lives feature-on-partitions: cur_k^T [128, R] etc.
  * z_k^T [(f,g), R] is built k-tile by k-tile on DVE tensor_tensor (bf16 2x):
      z-tile_f = cur_{k-1}^T * bcast(x0^T[f, :])
    The broadcast tiles come from one DRAM->SBUF DMA per chunk with a
    0-stride partition AP (DVE cannot partition-broadcast; DMA can).
  * The (f,g) contraction is standard PSUM-accumulated matmuls with weight
    k-tiles stationary, so the f-sum is free.
  * Layer 0 uses the x (x) x symmetry: W0 is host-symmetrized to the upper
    triangle (k: 1521 -> 780, padded 896) and the two z factors are
    host-gathered index patterns of x^T (pure indexing, no arithmetic).
  * All DRAM operands are laid out chunk-major on the host so every DMA has
    large contiguous per-partition runs (big packets -> full DMA bandwidth).
"""

import sys

sys.path.insert(0, "/opt/trn_rl_repo")

import numpy as np
import ml_dtypes

import concourse.bass as bass
import concourse.mybir as mybir
from concourse import bacc
from concourse.tile import TileContext
from concourse.bass_utils import run_bass_kernel_spmd

BF16 = ml_dtypes.bfloat16

B, F0, D = 2048, 39, 16
M = 128                      # layer width (all three layers)
NCORES = 8
BPC = B // NCORES            # batch per core = 256
R = BPC * D                  # rows per core = 4096
K0 = (F0 * (F0 + 1)) // 2    # 780 (triangular)
K0P = 896                    # padded to 7 k-tiles
NKT0 = K0P // 128            # 7
NKT = (F0 * M) // 128        # 39 k-tiles for layers 1/2

L = 512                      # bd-chunk (32 b x 16 d)
NCHUNK = R // L              # 8
BPCH = L // D                # 32 batches per chunk

DT = mybir.dt.bfloat16
DTF = mybir.dt.float32

_CACHE = {}


def _build_program():
    nc = bacc.Bacc("TRN2", target_bir_lowering=False, debug=False,
                   num_devices=NCORES)

    # chunk-major layouts so per-partition DMA runs are contiguous
    xT = nc.declare_dram_parameter("xT", [NCHUNK, F0 * L], DT, isOutput=False)
    zin0 = nc.declare_dram_parameter("zin0", [NCHUNK, 128, NKT0, L], DT,
                                     isOutput=False)
    zin1 = nc.declare_dram_parameter("zin1", [NCHUNK, 128, NKT0, L], DT,
                                     isOutput=False)
    w0 = nc.declare_dram_parameter("w0", [K0P, M], DT, isOutput=False)
    w1 = nc.declare_dram_parameter("w1", [F0 * M, M], DT, isOutput=False)
    w2 = nc.declare_dram_parameter("w2", [F0 * M, M], DT, isOutput=False)
    ident = nc.declare_dram_parameter("ident", [128, 128], DTF, isOutput=False)
    out = nc.declare_dram_parameter("out", [BPC, 3 * M], DTF, isOutput=True)

    with TileContext(nc) as tc:
        with (
            tc.tile_pool(name="wpool", bufs=1) as wpool,
            tc.tile_pool(name="bcast", bufs=2) as bcpool,
            tc.tile_pool(name="zin", bufs=2) as zinpool,
            tc.tile_pool(name="zt", bufs=3) as zpool,
            tc.tile_pool(name="cur", bufs=2) as curpool,
            tc.tile_pool(name="outp", bufs=1) as outpool,
            tc.tile_pool(name="psum", bufs=4, space="PSUM") as pspool,
            tc.tile_pool(name="pst", bufs=2, space="PSUM") as pstpool,
        ):
            def issue_chunk_dmas(c, pieces=1):
                """Prefetch one chunk's layer-0 factors + broadcast set.
                pieces>1 splits the transfers so the first compute group can
                start before the whole chunk arrives (startup only)."""
                zin0t = zinpool.tile([128, NKT0, L], DT, tag="zin0",
                                     name=f"zin0_{c}")
                zin1t = zinpool.tile([128, NKT0, L], DT, tag="zin1",
                                     name=f"zin1_{c}")
                if pieces == 1:
                    nc.scalar.dma_start(out=zin0t[:], in_=zin0[c])
                    nc.scalar.dma_start(out=zin1t[:], in_=zin1[c])
                else:
                    kt = 0
                    for gsz in G0:
                        nc.scalar.dma_start(
                            out=zin0t[:, kt : kt + gsz, :],
                            in_=zin0[c, :, kt : kt + gsz, :],
                        )
                        nc.scalar.dma_start(
                            out=zin1t[:, kt : kt + gsz, :],
                            in_=zin1[c, :, kt : kt + gsz, :],
                        )
                        kt += gsz
                bc = bcpool.tile([128, F0, L], DT, tag="bc", name=f"bc_{c}")
                xTv = xT[c : c + 1, :].to_broadcast((128, F0 * L)).rearrange(
                    "p (f l) -> p f l", f=F0
                )
                if pieces == 1:
                    nc.gpsimd.dma_start(out=bc[:], in_=xTv)
                else:
                    f = 0
                    for gsz in G12:
                        nc.gpsimd.dma_start(
                            out=bc[:, f : f + gsz, :], in_=xTv[:, f : f + gsz, :]
                        )
                        f += gsz
                return bc, zin0t, zin1t

            G0 = [4, 3]          # layer-0 k-tile TT groups (sum NKT0)
            G12 = [8, 8, 8, 8, 7]  # layer-1/2 f-groups (sum F0)

            # ---- small weights first (L0's matmuls need w0 immediately),
            # then the chunk-0 prefetch; w1/w2 follow on the sync queue.
            w0s = wpool.tile([128, NKT0, M], DT, tag="w0")
            nc.sync.dma_start(out=w0s[:], in_=w0.rearrange("(t p) m -> p t m", p=128))
            ids = wpool.tile([128, 128], DTF, tag="ident")
            nc.sync.dma_start(out=ids[:], in_=ident[:])
            pref = issue_chunk_dmas(0, pieces=2)
            w1s = wpool.tile([128, NKT, M], DT, tag="w1")
            nc.sync.dma_start(out=w1s[:], in_=w1.rearrange("(t p) m -> p t m", p=128))
            w2s = wpool.tile([128, NKT, M], DT, tag="w2")
            nc.sync.dma_start(out=w2s[:], in_=w2.rearrange("(t p) m -> p t m", p=128))

            # per-layer output accumulators [128 m, BPC] fp32
            outacc = [
                outpool.tile([128, BPC], DTF, tag=f"oacc{k}", name=f"oacc{k}")
                for k in range(3)
            ]

            def do_l0(c, zin0t, zin1t):
                """Layer 0 for chunk c: z0 = zin0*zin1, matmul -> cur0 (sbuf)."""
                ps0 = pspool.tile([128, L], DTF, tag="ps", name=f"ps0_{c}")
                kt = 0
                for gsz in G0:
                    z0t = zpool.tile([128, gsz, L], DT, tag="z",
                                     name=f"z0_{c}_{kt}")
                    nc.vector.tensor_mul(
                        z0t[:],
                        zin0t[:, kt : kt + gsz, :],
                        zin1t[:, kt : kt + gsz, :],
                    )
                    for j in range(gsz):
                        nc.tensor.matmul(
                            ps0[:],
                            w0s[:, kt + j, :],
                            z0t[:, j, :],
                            start=(kt + j == 0),
                            stop=(kt + j == NKT0 - 1),
                        )
                    kt += gsz
                cur0 = curpool.tile([128, L], DT, tag="cur0", name=f"cur0_{c}")
                nc.scalar.copy(cur0[:], ps0[:])
                nc.vector.tensor_reduce(
                    outacc[0][:, c * BPCH : (c + 1) * BPCH],
                    ps0[:].rearrange("p (b d) -> p b d", d=D),
                    axis=mybir.AxisListType.X,
                    op=mybir.AluOpType.add,
                )
                return cur0

            def do_layer(c, lyr, ws, prev, bc):
                """Layer 1/2 for chunk c: z = prev (x) bc, matmul, reduce."""
                ps = pspool.tile([128, L], DTF, tag="ps", name=f"ps{lyr}_{c}")
                f = 0
                for gsz in G12:
                    zt = zpool.tile([128, gsz, L], DT, tag="z",
                                    name=f"z{lyr}_{c}_{f}")
                    nc.vector.tensor_mul(
                        zt[:],
                        prev[:].unsqueeze(1).to_broadcast((128, gsz, L)),
                        bc[:, f : f + gsz, :],
                    )
                    for j in range(gsz):
                        nc.tensor.matmul(
                            ps[:],
                            ws[:, f + j, :],
                            zt[:, j, :],
                            start=(f + j == 0),
                            stop=(f + j == F0 - 1),
                        )
                    f += gsz
                nc.vector.tensor_reduce(
                    outacc[lyr][:, c * BPCH : (c + 1) * BPCH],
                    ps[:].rearrange("p (b d) -> p b d", d=D),
                    axis=mybir.AxisListType.X,
                    op=mybir.AluOpType.add,
                )
                if lyr == 1:
                    cur1 = curpool.tile([128, L], DT, tag="cur1",
                                        name=f"cur1_{c}")
                    nc.scalar.copy(cur1[:], ps[:])
                    return cur1
                return None

            # software pipeline: chunk c+1's L0 runs between chunk c's L1 and
            # L2 so the PE/DVE never drain at chunk boundaries.
            bc_c, zin0_c, zin1_c = pref
            cur0_c = do_l0(0, zin0_c, zin1_c)
            for c in range(NCHUNK):
                if c + 1 < NCHUNK:
                    pref = issue_chunk_dmas(c + 1)
                cur1_c = do_layer(c, 1, w1s, cur0_c, bc_c)
                if c + 1 < NCHUNK:
                    bc_n, zin0_n, zin1_n = pref
                    cur0_c = do_l0(c + 1, zin0_n, zin1_n)
                do_layer(c, 2, w2s, cur1_c, bc_c)
                if c + 1 < NCHUNK:
                    bc_c = bc_n

            # ---------- transpose [128 m, BPC b] -> [BPC, 128] and store ----------
            for k in range(3):
                for h in range(BPC // 128):
                    pst = pstpool.tile([128, 128], DTF, tag="pst")
                    nc.tensor.transpose(
                        pst[:], outacc[k][:, h * 128 : (h + 1) * 128], ids[:]
                    )
                    ot = curpool.tile([128, 128], DTF, tag="otile")
                    nc.scalar.copy(ot[:], pst[:])
                    nc.sync.dma_start(
                        out=out[h * 128 : (h + 1) * 128, k * M : (k + 1) * M],
                        in_=ot[:],
                    )

    nc.compile()
    return nc


def _host_prep(inputs, f0, f1, f2):
    """Per-core input maps. Pure layout/cast/index-gather, no FLOP offload
    (except the W0 symmetrization, which is weight preprocessing)."""
    x = np.asarray(inputs)

    # symmetrized triangular W0: rows (f, g) f<=g
    f0n = np.asarray(f0).reshape(F0, F0, M)
    fi, gi = np.triu_indices(F0)
    w0t = f0n[fi, gi] + np.where((fi != gi)[:, None], f0n[gi, fi], 0.0)
    w0 = np.zeros((K0P, M), dtype=BF16)
    w0[:K0] = w0t.astype(BF16)

    w1 = np.asarray(f1).astype(BF16)
    w2 = np.asarray(f2).astype(BF16)
    ident = np.eye(128, dtype=np.float32)

    # layer-0 z-factor gather indices (triangular, k-row = tile*128 + p)
    pidx = np.arange(K0P)
    fidx = np.zeros(K0P, np.int64)
    gidx = np.zeros(K0P, np.int64)
    fidx[:K0], gidx[:K0] = fi, gi
    valid = (pidx < K0).astype(BF16)[:, None]

    maps = []
    for c in range(NCORES):
        xs = x[c * BPC : (c + 1) * BPC]                    # [256, 39, 16]
        xTf = np.ascontiguousarray(
            xs.transpose(1, 0, 2).reshape(F0, R)
        ).astype(BF16)                                     # [39, R]
        # chunk-major broadcast source: [NCHUNK, F0*L]
        xTc = np.ascontiguousarray(
            xTf.reshape(F0, NCHUNK, L).transpose(1, 0, 2)
        ).reshape(NCHUNK, F0 * L)
        # layer-0 factors [K0P, R] -> chunk-major [NCHUNK, 128, NKT0, L]
        z0a = (xTf[gidx] * valid).reshape(NKT0, 128, NCHUNK, L)
        z0b = (xTf[fidx] * valid).reshape(NKT0, 128, NCHUNK, L)
        zin0c = np.ascontiguousarray(z0a.transpose(2, 1, 0, 3))
        zin1c = np.ascontiguousarray(z0b.transpose(2, 1, 0, 3))
        maps.append(
            dict(xT=xTc, zin0=zin0c, zin1=zin1c, w0=w0, w1=w1, w2=w2,
                 ident=ident)
        )
    return maps


def kernel(**inputs) -> np.ndarray:
    if "nc" not in _CACHE:
        _CACHE["nc"] = _build_program()
    nc = _CACHE["nc"]
    maps = _host_prep(inputs["inputs"], inputs["f0"], inputs["f1"], inputs["f2"])
    res = run_bass_kernel_spmd(nc, maps, list(range(NCORES)))
    return np.concatenate([res.results[c]["out"] for c in range(NCORES)], axis=0)


if __name__ == "__main__":
    rng = np.random.default_rng(0)
    ins = {
        "inputs": rng.standard_normal((B, F0, D), dtype=np.float32),
        "f0": (rng.standard_normal((F0 * F0, M)) * 0.05).astype(np.float32),
        "f1": (rng.standard_normal((F0 * M, M)) * 0.05).astype(np.float32),
        "f2": (rng.standard_normal((F0 * M, M)) * 0.05).astype(np.float32),
    }
    out = kernel(**ins)
    print("out", out.shape, out.dtype)


# revision 10
# speedup vs baseline: 1.3994x; 1.2488x over previous
"""CIN (Compressed Interaction Network) kernel for Trainium2, 8-core data parallel.

Math (per batch row b, embedding dim d — R = B*D independent rows):
  layer k: cur_k[m, (b,d)] = sum_{f,g} W_k[f*G+g, m] * x0[f,(b,d)] * x_{k}[g,(b,d)]
  output  = concat_k( sum_d cur_k )    -> [B, 384]

Device strategy (per core, batch-sharded B/8 = 256 -> R = 4096 rows):
  * Everything lives feature-on-partitions: cur_k^T [128, R] etc.
  * z_k^T [(f,g), R] is built k-tile by k-tile on DVE tensor_tensor (bf16 2x):
      z-tile_f = cur_{k-1}^T * bcast(x0^T[f, :])
    The broadcast tiles come from one DRAM->SBUF DMA per chunk with a
    0-stride partition AP (DVE cannot partition-broadcast; DMA can).
  * The (f,g) contraction is standard PSUM-accumulated matmuls with weight
    k-tiles stationary, so the f-sum is free.
  * Layer 0 uses the x (x) x symmetry: W0 is host-symmetrized to the upper
    triangle (k: 1521 -> 780, padded 896) and the two z factors are
    host-gathered index patterns of x^T (pure indexing, no arithmetic).
  * All DRAM operands are laid out chunk-major on the host so every DMA has
    large contiguous per-partition runs (big packets -> full DMA bandwidth).
"""

import sys

sys.path.insert(0, "/opt/trn_rl_repo")

import numpy as np
import ml_dtypes

import concourse.bass as bass
import concourse.mybir as mybir
from concourse import bacc
from concourse.tile import TileContext
from concourse.bass_utils import run_bass_kernel_spmd

BF16 = ml_dtypes.bfloat16

B, F0, D = 2048, 39, 16
M = 128                      # layer width (all three layers)
NCORES = 8
BPC = B // NCORES            # batch per core = 256
R = BPC * D                  # rows per core = 4096
K0 = (F0 * (F0 + 1)) // 2    # 780 (triangular)
K0P = 896                    # padded to 7 k-tiles
NKT0 = K0P // 128            # 7
NKT = (F0 * M) // 128        # 39 k-tiles for layers 1/2

L = 512                      # bd-chunk (32 b x 16 d)
NCHUNK = R // L              # 8
BPCH = L // D                # 32 batches per chunk

DT = mybir.dt.bfloat16
DTF = mybir.dt.float32

_CACHE = {}


def _build_program():
    nc = bacc.Bacc("TRN2", target_bir_lowering=False, debug=False,
                   num_devices=NCORES)

    # chunk-major layouts so per-partition DMA runs are contiguous
    xT = nc.declare_dram_parameter("xT", [NCHUNK, F0 * L], DT, isOutput=False)
    zin0 = nc.declare_dram_parameter("zin0", [NCHUNK, 128, NKT0, L], DT,
                                     isOutput=False)
    zin1 = nc.declare_dram_parameter("zin1", [NCHUNK, 128, NKT0, L], DT,
                                     isOutput=False)
    w0 = nc.declare_dram_parameter("w0", [K0P, M], DT, isOutput=False)
    w1 = nc.declare_dram_parameter("w1", [F0 * M, M], DT, isOutput=False)
    w2 = nc.declare_dram_parameter("w2", [F0 * M, M], DT, isOutput=False)
    ident = nc.declare_dram_parameter("ident", [128, 128], DTF, isOutput=False)
    out = nc.declare_dram_parameter("out", [BPC, 3 * M], DTF, isOutput=True)

    with TileContext(nc) as tc:
        with (
            tc.tile_pool(name="wpool", bufs=1) as wpool,
            tc.tile_pool(name="bcast", bufs=2) as bcpool,
            tc.tile_pool(name="zin", bufs=2) as zinpool,
            tc.tile_pool(name="zt", bufs=3) as zpool,
            tc.tile_pool(name="cur", bufs=2) as curpool,
            tc.tile_pool(name="outp", bufs=1) as outpool,
            tc.tile_pool(name="psum", bufs=4, space="PSUM") as pspool,
            tc.tile_pool(name="pst", bufs=2, space="PSUM") as pstpool,
        ):
            def issue_chunk_dmas(c, pieces=1):
                """Prefetch one chunk's layer-0 factors + broadcast set.
                pieces>1 splits the transfers so the first compute group can
                start before the whole chunk arrives (startup only)."""
                zin0t = zinpool.tile([128, NKT0, L], DT, tag="zin0",
                                     name=f"zin0_{c}")
                zin1t = zinpool.tile([128, NKT0, L], DT, tag="zin1",
                                     name=f"zin1_{c}")
                nc.sync.dma_start(out=zin0t[:], in_=zin0[c])
                nc.scalar.dma_start(out=zin1t[:], in_=zin1[c])
                bc = bcpool.tile([128, F0, L], DT, tag="bc", name=f"bc_{c}")
                xTv = xT[c : c + 1, :].to_broadcast((128, F0 * L)).rearrange(
                    "p (f l) -> p f l", f=F0
                )
                h = F0 // 2
                nc.sync.dma_start(out=bc[:, :h, :], in_=xTv[:, :h, :])
                nc.scalar.dma_start(out=bc[:, h:, :], in_=xTv[:, h:, :])
                return bc, zin0t, zin1t

            G0 = [4, 3]          # layer-0 k-tile TT groups (sum NKT0)
            G12 = [8, 8, 8, 8, 7]  # layer-1/2 f-groups (sum F0)

            # ---- small weights first (L0's matmuls need w0 immediately),
            # then the chunk-0 prefetch; w1/w2 follow on the sync queue.
            w0s = wpool.tile([128, NKT0, M], DT, tag="w0")
            nc.sync.dma_start(out=w0s[:], in_=w0.rearrange("(t p) m -> p t m", p=128))
            ids = wpool.tile([128, 128], DTF, tag="ident")
            nc.sync.dma_start(out=ids[:], in_=ident[:])
            pref = issue_chunk_dmas(0)
            w1s = wpool.tile([128, NKT, M], DT, tag="w1")
            nc.gpsimd.dma_start(out=w1s[:], in_=w1.rearrange("(t p) m -> p t m", p=128))
            w2s = wpool.tile([128, NKT, M], DT, tag="w2")
            nc.gpsimd.dma_start(out=w2s[:], in_=w2.rearrange("(t p) m -> p t m", p=128))

            # per-layer output accumulators [128 m, BPC] fp32
            outacc = [
                outpool.tile([128, BPC], DTF, tag=f"oacc{k}", name=f"oacc{k}")
                for k in range(3)
            ]

            def do_l0(c, zin0t, zin1t):
                """Layer 0 for chunk c: z0 = zin0*zin1, matmul -> cur0 (sbuf)."""
                ps0 = pspool.tile([128, L], DTF, tag="ps", name=f"ps0_{c}")
                kt = 0
                for gsz in G0:
                    z0t = zpool.tile([128, gsz, L], DT, tag="z",
                                     name=f"z0_{c}_{kt}")
                    nc.vector.tensor_mul(
                        z0t[:],
                        zin0t[:, kt : kt + gsz, :],
                        zin1t[:, kt : kt + gsz, :],
                    )
                    for j in range(gsz):
                        nc.tensor.matmul(
                            ps0[:],
                            w0s[:, kt + j, :],
                            z0t[:, j, :],
                            start=(kt + j == 0),
                            stop=(kt + j == NKT0 - 1),
                        )
                    kt += gsz
                cur0 = curpool.tile([128, L], DT, tag="cur0", name=f"cur0_{c}")
                nc.scalar.copy(cur0[:], ps0[:])
                nc.vector.tensor_reduce(
                    outacc[0][:, c * BPCH : (c + 1) * BPCH],
                    ps0[:].rearrange("p (b d) -> p b d", d=D),
                    axis=mybir.AxisListType.X,
                    op=mybir.AluOpType.add,
                )
                return cur0

            def do_layer(c, lyr, ws, prev, bc):
                """Layer 1/2 for chunk c: z = prev (x) bc, matmul, reduce."""
                ps = pspool.tile([128, L], DTF, tag="ps", name=f"ps{lyr}_{c}")
                f = 0
                for gsz in G12:
                    zt = zpool.tile([128, gsz, L], DT, tag="z",
                                    name=f"z{lyr}_{c}_{f}")
                    nc.vector.tensor_mul(
                        zt[:],
                        prev[:].unsqueeze(1).to_broadcast((128, gsz, L)),
                        bc[:, f : f + gsz, :],
                    )
                    for j in range(gsz):
                        nc.tensor.matmul(
                            ps[:],
                            ws[:, f + j, :],
                            zt[:, j, :],
                            start=(f + j == 0),
                            stop=(f + j == F0 - 1),
                        )
                    f += gsz
                nc.vector.tensor_reduce(
                    outacc[lyr][:, c * BPCH : (c + 1) * BPCH],
                    ps[:].rearrange("p (b d) -> p b d", d=D),
                    axis=mybir.AxisListType.X,
                    op=mybir.AluOpType.add,
                )
                if lyr == 1:
                    cur1 = curpool.tile([128, L], DT, tag="cur1",
                                        name=f"cur1_{c}")
                    nc.scalar.copy(cur1[:], ps[:])
                    return cur1
                return None

            # software pipeline: chunk c+1's L0 runs between chunk c's L1 and
            # L2 so the PE/DVE never drain at chunk boundaries.
            bc_c, zin0_c, zin1_c = pref
            cur0_c = do_l0(0, zin0_c, zin1_c)
            for c in range(NCHUNK):
                cur1_c = do_layer(c, 1, w1s, cur0_c, bc_c)
                if c + 1 < NCHUNK:
                    bc_n, zin0_n, zin1_n = issue_chunk_dmas(c + 1)
                    cur0_c = do_l0(c + 1, zin0_n, zin1_n)
                do_layer(c, 2, w2s, cur1_c, bc_c)
                if c + 1 < NCHUNK:
                    bc_c = bc_n

            # ---------- transpose [128 m, BPC b] -> [BPC, 128] and store ----------
            for k in range(3):
                for h in range(BPC // 128):
                    pst = pstpool.tile([128, 128], DTF, tag="pst")
                    nc.tensor.transpose(
                        pst[:], outacc[k][:, h * 128 : (h + 1) * 128], ids[:]
                    )
                    ot = curpool.tile([128, 128], DTF, tag="otile")
                    nc.scalar.copy(ot[:], pst[:])
                    nc.sync.dma_start(
                        out=out[h * 128 : (h + 1) * 128, k * M : (k + 1) * M],
                        in_=ot[:],
                    )

    nc.compile()
    return nc


def _host_prep(inputs, f0, f1, f2):
    """Per-core input maps. Pure layout/cast/index-gather, no FLOP offload
    (except the W0 symmetrization, which is weight preprocessing)."""
    x = np.asarray(inputs)

    # symmetrized triangular W0: rows (f, g) f<=g
    f0n = np.asarray(f0).reshape(F0, F0, M)
    fi, gi = np.triu_indices(F0)
    w0t = f0n[fi, gi] + np.where((fi != gi)[:, None], f0n[gi, fi], 0.0)
    w0 = np.zeros((K0P, M), dtype=BF16)
    w0[:K0] = w0t.astype(BF16)

    w1 = np.asarray(f1).astype(BF16)
    w2 = np.asarray(f2).astype(BF16)
    ident = np.eye(128, dtype=np.float32)

    # layer-0 z-factor gather indices (triangular, k-row = tile*128 + p)
    pidx = np.arange(K0P)
    fidx = np.zeros(K0P, np.int64)
    gidx = np.zeros(K0P, np.int64)
    fidx[:K0], gidx[:K0] = fi, gi
    valid = (pidx < K0).astype(BF16)[:, None]

    maps = []
    for c in range(NCORES):
        xs = x[c * BPC : (c + 1) * BPC]                    # [256, 39, 16]
        xTf = np.ascontiguousarray(
            xs.transpose(1, 0, 2).reshape(F0, R)
        ).astype(BF16)                                     # [39, R]
        # chunk-major broadcast source: [NCHUNK, F0*L]
        xTc = np.ascontiguousarray(
            xTf.reshape(F0, NCHUNK, L).transpose(1, 0, 2)
        ).reshape(NCHUNK, F0 * L)
        # layer-0 factors [K0P, R] -> chunk-major [NCHUNK, 128, NKT0, L]
        z0a = (xTf[gidx] * valid).reshape(NKT0, 128, NCHUNK, L)
        z0b = (xTf[fidx] * valid).reshape(NKT0, 128, NCHUNK, L)
        zin0c = np.ascontiguousarray(z0a.transpose(2, 1, 0, 3))
        zin1c = np.ascontiguousarray(z0b.transpose(2, 1, 0, 3))
        maps.append(
            dict(xT=xTc, zin0=zin0c, zin1=zin1c, w0=w0, w1=w1, w2=w2,
                 ident=ident)
        )
    return maps


def kernel(**inputs) -> np.ndarray:
    if "nc" not in _CACHE:
        _CACHE["nc"] = _build_program()
    nc = _CACHE["nc"]
    maps = _host_prep(inputs["inputs"], inputs["f0"], inputs["f1"], inputs["f2"])
    res = run_bass_kernel_spmd(nc, maps, list(range(NCORES)))
    return np.concatenate([res.results[c]["out"] for c in range(NCORES)], axis=0)


if __name__ == "__main__":
    rng = np.random.default_rng(0)
    ins = {
        "inputs": rng.standard_normal((B, F0, D), dtype=np.float32),
        "f0": (rng.standard_normal((F0 * F0, M)) * 0.05).astype(np.float32),
        "f1": (rng.standard_normal((F0 * M, M)) * 0.05).astype(np.float32),
        "f2": (rng.standard_normal((F0 * M, M)) * 0.05).astype(np.float32),
    }
    out = kernel(**ins)
    print("out", out.shape, out.dtype)


# revision 11
# speedup vs baseline: 1.5134x; 1.0814x over previous
"""CIN (Compressed Interaction Network) kernel for Trainium2, 8-core data parallel.

Math (per batch row b, embedding dim d — R = B*D independent rows):
  layer k: cur_k[m, (b,d)] = sum_{f,g} W_k[f*G+g, m] * x0[f,(b,d)] * x_{k}[g,(b,d)]
  output  = concat_k( sum_d cur_k )    -> [B, 384]

Device strategy (per core, batch-sharded B/8 = 256 -> R = 4096 rows):
  * Everything lives feature-on-partitions: cur_k^T [128, R] etc.
  * z_k^T [(f,g), R] is built k-tile by k-tile on DVE tensor_tensor (bf16 2x):
      z-tile_f = cur_{k-1}^T * bcast(x0^T[f, :])
    The broadcast tiles come from one DRAM->SBUF DMA per chunk with a
    0-stride partition AP (DVE cannot partition-broadcast; DMA can).
  * The (f,g) contraction is standard PSUM-accumulated matmuls with weight
    k-tiles stationary, so the f-sum is free.
  * Layer 0 uses the x (x) x symmetry: W0 is host-symmetrized to the upper
    triangle (k: 1521 -> 780, padded 896) and the two z factors are
    host-gathered index patterns of x^T (pure indexing, no arithmetic).
  * All DRAM operands are laid out chunk-major on the host so every DMA has
    large contiguous per-partition runs (big packets -> full DMA bandwidth).
"""

import sys

sys.path.insert(0, "/opt/trn_rl_repo")

import numpy as np
import ml_dtypes

import concourse.bass as bass
import concourse.mybir as mybir
from concourse import bacc
from concourse.tile import TileContext
from concourse.bass_utils import run_bass_kernel_spmd

BF16 = ml_dtypes.bfloat16

B, F0, D = 2048, 39, 16
M = 128                      # layer width (all three layers)
NCORES = 8
BPC = B // NCORES            # batch per core = 256
R = BPC * D                  # rows per core = 4096
K0 = (F0 * (F0 + 1)) // 2    # 780 (triangular)
K0P = 896                    # padded to 7 k-tiles
NKT0 = K0P // 128            # 7
NKT = (F0 * M) // 128        # 39 k-tiles for layers 1/2

L = 512                      # bd-chunk (32 b x 16 d)
NCHUNK = R // L              # 8
BPCH = L // D                # 32 batches per chunk
NTILE = L // 128             # 4 bd-tiles of 128 rows per chunk

DT = mybir.dt.bfloat16
DTF = mybir.dt.float32

_CACHE = {}


def _build_program():
    nc = bacc.Bacc("TRN2", target_bir_lowering=False, debug=False,
                   num_devices=NCORES)

    # chunk-major layouts so per-partition DMA runs are contiguous
    xT = nc.declare_dram_parameter("xT", [NCHUNK, F0 * L], DT, isOutput=False)
    zin0 = nc.declare_dram_parameter("zin0", [NCHUNK, 128, NKT0, L], DT,
                                     isOutput=False)
    zin1 = nc.declare_dram_parameter("zin1", [NCHUNK, 128, NKT0, L], DT,
                                     isOutput=False)
    w0 = nc.declare_dram_parameter("w0", [K0P, M], DT, isOutput=False)
    w1 = nc.declare_dram_parameter("w1", [F0 * M, M], DT, isOutput=False)
    w2 = nc.declare_dram_parameter("w2", [F0 * M, M], DT, isOutput=False)
    ident = nc.declare_dram_parameter("ident", [128, 128], DTF, isOutput=False)
    identb = nc.declare_dram_parameter("identb", [128, 128], DT, isOutput=False)
    xbd = nc.declare_dram_parameter("xbd", [NCHUNK, 128, NTILE, F0], DT,
                                    isOutput=False)
    mask3 = nc.declare_dram_parameter("mask3", [128, 9 * F0], DT,
                                      isOutput=False)
    out = nc.declare_dram_parameter("out", [BPC, 3 * M], DTF, isOutput=True)

    with TileContext(nc) as tc:
        with (
            tc.tile_pool(name="wpool", bufs=1) as wpool,
            tc.tile_pool(name="bcast", bufs=2) as bcpool,
            tc.tile_pool(name="zin", bufs=2) as zinpool,
            tc.tile_pool(name="zt", bufs=3) as zpool,
            tc.tile_pool(name="cur", bufs=2) as curpool,
            tc.tile_pool(name="outp", bufs=1) as outpool,
            tc.tile_pool(name="psum", bufs=3, space="PSUM") as pspool,
            tc.tile_pool(name="pst", bufs=1, space="PSUM") as pstpool,
            tc.tile_pool(name="pstb", bufs=2, space="PSUM") as pstbpool,
            tc.tile_pool(name="psa", bufs=1, space="PSUM") as psapool,
            tc.tile_pool(name="pso2", bufs=1, space="PSUM") as pso2pool,
        ):
            def issue_chunk_dmas(c, pieces=1):
                """Prefetch one chunk's layer-0 factors + broadcast set.
                pieces>1 splits the transfers so the first compute group can
                start before the whole chunk arrives (startup only)."""
                zin0t = zinpool.tile([128, NKT0, L], DT, tag="zin0",
                                     name=f"zin0_{c}")
                zin1t = zinpool.tile([128, NKT0, L], DT, tag="zin1",
                                     name=f"zin1_{c}")
                if c == 0:
                    nc.sync.dma_start(out=zin0t[:, :4, :], in_=zin0[c, :, :4, :])
                    nc.scalar.dma_start(out=zin1t[:, :4, :], in_=zin1[c, :, :4, :])
                    nc.sync.dma_start(out=zin0t[:, 4:, :], in_=zin0[c, :, 4:, :])
                    nc.scalar.dma_start(out=zin1t[:, 4:, :], in_=zin1[c, :, 4:, :])
                else:
                    nc.sync.dma_start(out=zin0t[:], in_=zin0[c])
                    nc.scalar.dma_start(out=zin1t[:], in_=zin1[c])
                bc = bcpool.tile([128, F0, L], DT, tag="bc", name=f"bc_{c}")
                xTv = xT[c : c + 1, :].to_broadcast((128, F0 * L)).rearrange(
                    "p (f l) -> p f l", f=F0
                )
                h = F0 // 2
                nc.sync.dma_start(out=bc[:, :h, :], in_=xTv[:, :h, :])
                nc.scalar.dma_start(out=bc[:, h:, :], in_=xTv[:, h:, :])
                return bc, zin0t, zin1t

            G0 = [4, 3]          # layer-0 k-tile TT groups (sum NKT0)
            G12 = [8, 8, 8, 8, 7]  # layer-1/2 f-groups (sum F0)

            # ---- small weights first (L0's matmuls need w0 immediately),
            # then the chunk-0 prefetch; w1/w2 follow on the sync queue.
            w0s = wpool.tile([128, NKT0, M], DT, tag="w0")
            nc.sync.dma_start(out=w0s[:], in_=w0.rearrange("(t p) m -> p t m", p=128))
            ids = wpool.tile([128, 128], DTF, tag="ident")
            nc.sync.dma_start(out=ids[:], in_=ident[:])
            idb = wpool.tile([128, 128], DT, tag="identb")
            nc.sync.dma_start(out=idb[:], in_=identb[:])
            m3 = wpool.tile([128, 9, F0], DT, tag="mask3")
            nc.sync.dma_start(out=m3[:], in_=mask3.rearrange("p (q f) -> p q f", f=F0))
            pref = issue_chunk_dmas(0)
            w1s = wpool.tile([128, NKT, M], DT, tag="w1")
            nc.gpsimd.dma_start(out=w1s[:], in_=w1.rearrange("(t p) m -> p t m", p=128))
            w2s = wpool.tile([128, NKT, M], DT, tag="w2")
            nc.gpsimd.dma_start(out=w2s[:], in_=w2.rearrange("(t p) m -> p t m", p=128))

            # per-layer output accumulators [128 m, BPC] fp32
            outacc = [
                outpool.tile([128, BPC], DTF, tag=f"oacc{k}", name=f"oacc{k}")
                for k in range(3)
            ]

            def do_l0(c, zin0t, zin1t):
                """Layer 0 for chunk c: z0 = zin0*zin1, matmul -> cur0 (sbuf)."""
                ps0 = pspool.tile([128, L], DTF, tag="ps", name=f"ps0_{c}")
                kt = 0
                for gsz in G0:
                    z0t = zpool.tile([128, gsz, L], DT, tag="z",
                                     name=f"z0_{c}_{kt}")
                    nc.vector.tensor_mul(
                        z0t[:],
                        zin0t[:, kt : kt + gsz, :],
                        zin1t[:, kt : kt + gsz, :],
                    )
                    for j in range(gsz):
                        nc.tensor.matmul(
                            ps0[:],
                            w0s[:, kt + j, :],
                            z0t[:, j, :],
                            start=(kt + j == 0),
                            stop=(kt + j == NKT0 - 1),
                        )
                    kt += gsz
                cur0 = curpool.tile([128, L], DT, tag="cur0", name=f"cur0_{c}")
                nc.scalar.copy(cur0[:], ps0[:])
                nc.vector.tensor_reduce(
                    outacc[0][:, c * BPCH : (c + 1) * BPCH],
                    ps0[:].rearrange("p (b d) -> p b d", d=D),
                    axis=mybir.AxisListType.X,
                    op=mybir.AluOpType.add,
                )
                return cur0

            def do_layer(c, lyr, ws, prev, bc):
                """Layer 1/2 for chunk c: z = prev (x) bc, matmul, reduce."""
                ps = pspool.tile([128, L], DTF, tag="ps", name=f"ps{lyr}_{c}")
                f = 0
                for gsz in G12:
                    zt = zpool.tile([128, gsz, L], DT, tag="z",
                                    name=f"z{lyr}_{c}_{f}")
                    nc.vector.tensor_mul(
                        zt[:],
                        prev[:].unsqueeze(1).to_broadcast((128, gsz, L)),
                        bc[:, f : f + gsz, :],
                    )
                    for j in range(gsz):
                        nc.tensor.matmul(
                            ps[:],
                            ws[:, f + j, :],
                            zt[:, j, :],
                            start=(f + j == 0),
                            stop=(f + j == F0 - 1),
                        )
                    f += gsz
                nc.vector.tensor_reduce(
                    outacc[lyr][:, c * BPCH : (c + 1) * BPCH],
                    ps[:].rearrange("p (b d) -> p b d", d=D),
                    axis=mybir.AxisListType.X,
                    op=mybir.AluOpType.add,
                )
                if lyr == 1:
                    cur1 = curpool.tile([128, L], DT, tag="cur1",
                                        name=f"cur1_{c}")
                    nc.scalar.copy(cur1[:], ps[:])
                    return cur1
                return None

            # layer 2 via the d-contraction (Gram) trick: the final output
            # only needs sum_d cur2, and
            #   out2[b, m] = sum_{f,g} W2[fg, m] * P12[b, f, g],
            #   P12[b, f, g] = sum_d x[b, f, d] * cur1[b, g, d].
            # P12 is built on the PE with a block-diagonal x operand (bd rows
            # on partitions, contraction over the 16 d's of each batch).
            p12 = outpool.tile([128, NCHUNK * NTILE * 9 * F0], DT, tag="p12")

            def do_l2p(c, cur1, xbdt):
                for t in range(NTILE):
                    pstc = pstbpool.tile([128, 128], DT, tag="pstb",
                                        name=f"pstc_{c}_{t}")
                    nc.tensor.transpose(
                        pstc[:], cur1[:, t * 128 : (t + 1) * 128], idb[:]
                    )
                    c1bd = curpool.tile([128, 128], DT, tag="c1bd",
                                        name=f"c1bd_{c}_{t}")
                    nc.scalar.copy(c1bd[:], pstc[:])
                    bd3 = zpool.tile([128, 9, F0], DT, tag="bd3",
                                     name=f"bd3_{c}_{t}")
                    nc.vector.tensor_mul(
                        bd3[:],
                        xbdt[:, t, :].unsqueeze(1).to_broadcast((128, 9, F0)),
                        m3[:],
                    )
                    psa = psapool.tile([128, 9 * F0], DTF, tag="psa",
                                       name=f"psa_{c}_{t}")
                    nc.tensor.matmul(
                        psa[:], c1bd[:], bd3[:].rearrange("p q f -> p (q f)"),
                        start=True, stop=True,
                    )
                    off = (c * NTILE + t) * 9 * F0
                    nc.scalar.copy(p12[:, off : off + 9 * F0], psa[:])

            # stage B of the L2 Gram trick, over a half-range of chunks:
            # out2^T[m, b-half] = sum_f w2_f^T @ p12[:, (c, t, b, f)]
            pso2 = pso2pool.tile([128, BPC], DTF, tag="pso2")
            p12v = p12[:].rearrange("p (ct s f) -> p ct s f", s=9, f=F0)

            def stage_b(c0, c1):
                nt0, nt1 = c0 * NTILE, c1 * NTILE
                b0, b1 = c0 * BPCH, c1 * BPCH
                for f in range(F0):
                    nc.tensor.matmul(
                        pso2[:, b0:b1], w2s[:, f, :],
                        p12v[:, nt0:nt1, 0:8, f],
                        start=(f == 0), stop=(f == F0 - 1),
                    )
                nc.scalar.copy(outacc[2][:, b0:b1], pso2[:, b0:b1])

            # software pipeline: chunk c+1's L0 runs between chunk c's L1 and
            # the L2 P-build so the PE/DVE never drain at chunk boundaries.
            bc_c, zin0_c, zin1_c = pref
            xbdt_c = zinpool.tile([128, NTILE, F0], DT, tag="xbd", name="xbd_0")
            nc.scalar.dma_start(out=xbdt_c[:], in_=xbd[0])
            cur0_c = do_l0(0, zin0_c, zin1_c)
            for c in range(NCHUNK):
                cur1_c = do_layer(c, 1, w1s, cur0_c, bc_c)
                if c + 1 < NCHUNK:
                    bc_n, zin0_n, zin1_n = issue_chunk_dmas(c + 1)
                    xbdt_n = zinpool.tile([128, NTILE, F0], DT, tag="xbd",
                                          name=f"xbd_{c + 1}")
                    nc.scalar.dma_start(out=xbdt_n[:], in_=xbd[c + 1])
                    cur0_c = do_l0(c + 1, zin0_n, zin1_n)
                do_l2p(c, cur1_c, xbdt_c)
                if c == 3:
                    stage_b(0, 4)
                if c + 1 < NCHUNK:
                    bc_c, xbdt_c = bc_n, xbdt_n

            # stage B second half (chunks 4-7); first half was emitted inside
            # the chunk loop right after chunk 3's P-build.
            stage_b(4, 8)

            # ---------- transpose [128 m, BPC b] -> [BPC, 128] and store ----------
            for k in range(3):
                for h in range(BPC // 128):
                    pst = pstpool.tile([128, 128], DTF, tag="pst")
                    nc.tensor.transpose(
                        pst[:], outacc[k][:, h * 128 : (h + 1) * 128], ids[:]
                    )
                    ot = curpool.tile([128, 128], DTF, tag="otile")
                    nc.scalar.copy(ot[:], pst[:])
                    nc.sync.dma_start(
                        out=out[h * 128 : (h + 1) * 128, k * M : (k + 1) * M],
                        in_=ot[:],
                    )

    nc.compile()
    return nc


def _host_prep(inputs, f0, f1, f2):
    """Per-core input maps. Pure layout/cast/index-gather, no FLOP offload
    (except the W0 symmetrization, which is weight preprocessing)."""
    x = np.asarray(inputs)

    # symmetrized triangular W0: rows (f, g) f<=g
    f0n = np.asarray(f0).reshape(F0, F0, M)
    fi, gi = np.triu_indices(F0)
    w0t = f0n[fi, gi] + np.where((fi != gi)[:, None], f0n[gi, fi], 0.0)
    w0 = np.zeros((K0P, M), dtype=BF16)
    w0[:K0] = w0t.astype(BF16)

    w1 = np.asarray(f1).astype(BF16)
    w2 = np.asarray(f2).astype(BF16)
    ident = np.eye(128, dtype=np.float32)
    identb = np.eye(128, dtype=BF16)
    mask3 = np.zeros((128, 9 * F0), dtype=BF16)
    for p in range(128):
        s = p // 16
        mask3[p, s * F0 : (s + 1) * F0] = 1

    # layer-0 z-factor gather indices (triangular, k-row = tile*128 + p)
    pidx = np.arange(K0P)
    fidx = np.zeros(K0P, np.int64)
    gidx = np.zeros(K0P, np.int64)
    fidx[:K0], gidx[:K0] = fi, gi
    valid = (pidx < K0).astype(BF16)[:, None]

    maps = []
    for c in range(NCORES):
        xs = x[c * BPC : (c + 1) * BPC]                    # [256, 39, 16]
        xTf = np.ascontiguousarray(
            xs.transpose(1, 0, 2).reshape(F0, R)
        ).astype(BF16)                                     # [39, R]
        # chunk-major broadcast source: [NCHUNK, F0*L]
        xTc = np.ascontiguousarray(
            xTf.reshape(F0, NCHUNK, L).transpose(1, 0, 2)
        ).reshape(NCHUNK, F0 * L)
        # layer-0 factors [K0P, R] -> chunk-major [NCHUNK, 128, NKT0, L]
        z0a = (xTf[gidx] * valid).reshape(NKT0, 128, NCHUNK, L)
        z0b = (xTf[fidx] * valid).reshape(NKT0, 128, NCHUNK, L)
        zin0c = np.ascontiguousarray(z0a.transpose(2, 1, 0, 3))
        zin1c = np.ascontiguousarray(z0b.transpose(2, 1, 0, 3))
        xbd_full = xs.transpose(0, 2, 1).reshape(R, F0).astype(BF16)
        xbdh = np.ascontiguousarray(
            xbd_full.reshape(NCHUNK, NTILE, 128, F0).transpose(0, 2, 1, 3)
        )
        maps.append(
            dict(xT=xTc, zin0=zin0c, zin1=zin1c, w0=w0, w1=w1, w2=w2,
                 ident=ident, identb=identb, xbd=xbdh, mask3=mask3)
        )
    return maps


def kernel(**inputs) -> np.ndarray:
    if "nc" not in _CACHE:
        _CACHE["nc"] = _build_program()
    nc = _CACHE["nc"]
    maps = _host_prep(inputs["inputs"], inputs["f0"], inputs["f1"], inputs["f2"])
    res = run_bass_kernel_spmd(nc, maps, list(range(NCORES)))
    return np.concatenate([res.results[c]["out"] for c in range(NCORES)], axis=0)


if __name__ == "__main__":
    rng = np.random.default_rng(0)
    ins = {
        "inputs": rng.standard_normal((B, F0, D), dtype=np.float32),
        "f0": (rng.standard_normal((F0 * F0, M)) * 0.05).astype(np.float32),
        "f1": (rng.standard_normal((F0 * M, M)) * 0.05).astype(np.float32),
        "f2": (rng.standard_normal((F0 * M, M)) * 0.05).astype(np.float32),
    }
    out = kernel(**ins)
    print("out", out.shape, out.dtype)
